# revision 33
# baseline (speedup 1.0000x reference)
"""Trainium2 Bass kernel for int8-dequant Linear: out = x @ (W_q * scaler)^T.

Full shapes: x [4, 2048, 4096] f32, weight_q [4096, 4096] int8,
weight_scaler [4096] f32 -> out [4, 2048, 4096] f32.

Sharding: data-parallel over tokens (8192 tokens -> 1024 per core);
weight_q/scaler replicated. Each core computes out.T for its token
shard with out-channels on PSUM partitions; the per-channel scaler is
applied as a per-partition scalar multiply on PSUM eviction.

Matmul dtype: float32r (TF32-like fast fp32 mode, 1 cyc/row at free
dim >= 256; measured rel-err ~2e-4 on K=4096 accumulation).
Fallback MODE "bf16x2": W exact in bf16 (int8-valued), x split into
bf16 hi+lo, two accumulation passes (rel err ~5e-6, 2x matmul work).
"""
import sys

sys.path.insert(0, "/opt/trn_rl_repo")

import numpy as np

import concourse.bacc as bacc
import concourse.mybir as mybir
import concourse.tile as tile
from concourse.bass_utils import run_bass_kernel_spmd

N_CORES = 8
P = 128
IN_F = 4096
OUT_F = 4096
TOKENS = 4 * 2048
T_SHARD = TOKENS // N_CORES          # 1024 tokens per core
KT = IN_F // P                       # 32 k-tiles
MT = OUT_F // P                      # 32 m-tiles (out-channel tiles)
N_FREE = 512                         # moving free dim per matmul (1 PSUM bank)
NT = T_SHARD // N_FREE               # 2 n-tiles

MODE = "f32r"                        # "f32r" | "fp16" | "bf16x2"

_cache = {}


def _build(mode):
    f32 = mybir.dt.float32
    mm_dt = {
        "f32r": mybir.dt.float32r,
        "fp16": mybir.dt.float16,
        "bf16x2": mybir.dt.bfloat16,
    }[mode]
    n_pass = 2 if mode == "bf16x2" else 1

    nc = bacc.Bacc(None, target_bir_lowering=False, debug=False)

    # DRAM parameters (per-core shapes)
    d_x = [
        nc.declare_dram_parameter(f"xq{i}", [IN_F, T_SHARD], mm_dt, isOutput=False)
        for i in range(n_pass)
    ]
    # Weights travel as int8 (4x less DMA + in-window bandwidth) and are
    # upconverted to the matmul dtype on-chip by the otherwise-idle DVE.
    d_w = nc.declare_dram_parameter("wq4", [MT, P, KT, P], mybir.dt.int8, isOutput=False)
    d_s = nc.declare_dram_parameter("scal", [P, MT], f32, isOutput=False)
    d_o = nc.declare_dram_parameter("outT", [MT, P, T_SHARD], f32, isOutput=True)

    WH = 4                       # k-tiles per weight sub-tile
    NH = KT // WH                # weight sub-tiles per mo
    PRO = 4 if n_pass == 1 else 0  # mo-tiles interleaved during the x load

    with tile.TileContext(nc) as tc:
        with (
            tc.tile_pool(name="xp", bufs=KT * n_pass) as xp,
            tc.tile_pool(name="wp", bufs=16) as wp,
            tc.tile_pool(name="ws", bufs=6) as ws,
            tc.tile_pool(name="op", bufs=4) as op,
            tc.tile_pool(name="cp", bufs=1) as cp,
            tc.tile_pool(name="ps", bufs=8, space="PSUM") as ps,
        ):
            scal = cp.tile([P, MT], f32)
            nc.sync.dma_start(scal[:], d_s.ap())

            def w_half(mo, h):
                s = ws.tile([P, WH, P], mybir.dt.int8, tag="w8", name=f"w8_{mo}_{h}")
                nc.sync.dma_start(s[:], d_w.ap()[mo, :, h * WH:(h + 1) * WH, :])
                t = wp.tile([P, WH, P], mm_dt, tag="wh", name=f"wh_{mo}_{h}")
                nc.vector.tensor_copy(t[:], s[:])
                return t

            def x_tile(i):
                ip, k = divmod(i, KT)
                t = xp.tile([P, T_SHARD], mm_dt, tag="xt", name=f"xt_{i}")
                nc.sync.dma_start(t[:], d_x[ip].ap()[k * P:(k + 1) * P, :])
                return t

            def evict(mo, n, psum):
                osb = op.tile([P, N_FREE], f32, tag="osb", name=f"osb_{mo}_{n}")
                nc.vector.tensor_scalar_mul(osb[:], psum[:], scal[:, mo:mo + 1])
                nc.sync.dma_start(
                    d_o.ap()[mo, :, n * N_FREE:(n + 1) * N_FREE], osb[:]
                )

            nk = KT * n_pass
            xt = [None] * nk
            wh_pro = {}

            # --- phase 1: first PRO mo-tiles, k-major across mo so the PE
            # has work for every x k-tile as it lands. Each k-group's
            # weight quarters are emitted just ahead of that group's x
            # k-tiles; the last group also prefetches mo=PRO's quarters so
            # phase 2 starts without waiting behind the x tail.
            wh_next = {}
            if PRO:
                for h in range(NH):
                    if h == 0:
                        xt[0] = x_tile(0)
                        wh_pro[(0, 0)] = w_half(0, 0)
                        for mo in range(1, PRO):
                            wh_pro[(mo, 0)] = w_half(mo, 0)
                        for i in range(1, WH):
                            xt[i] = x_tile(i)
                    else:
                        for mo in range(PRO):
                            wh_pro[(mo, h)] = w_half(mo, h)
                        if h == NH - 1:
                            for j in range(NH):
                                wh_next[j] = w_half(PRO, j)
                        for i in range(h * WH, (h + 1) * WH):
                            xt[i] = x_tile(i)

                pro_ps = {
                    (mo, n): ps.tile(
                        [P, N_FREE], f32, tag="psum", name=f"psum_{mo}_{n}"
                    )
                    for mo in range(PRO)
                    for n in range(NT)
                }
                # Request mo=PRO's banks now so the allocator binds them to
                # the earliest-released phase-1 banks (shrinks the
                # phase-boundary stall).
                early_ps = [
                    ps.tile([P, N_FREE], f32, tag="psum", name=f"psum_{PRO}_{n}")
                    for n in range(NT)
                ]
                for i in range(nk):
                    ip, k = divmod(i, KT)
                    h, kh = divmod(k, WH)
                    for mo in range(PRO):
                        for n in range(NT):
                            nc.tensor.matmul(
                                pro_ps[(mo, n)][:],
                                wh_pro[(mo, h)][:, kh, :],
                                xt[i][:, n * N_FREE:(n + 1) * N_FREE],
                                start=(i == 0),
                                stop=(i == nk - 1),
                            )
                for mo in range(PRO):
                    for n in range(NT):
                        evict(mo, n, pro_ps[(mo, n)])
            else:
                early_ps = None
                for i in range(nk):
                    xt[i] = x_tile(i)

            # --- phase 2: remaining mo-tiles, weight-reuse-friendly order
            # (k middle, n inner).
            for mo in range(PRO, MT):
                if mo == PRO and wh_next:
                    whs = [wh_next[h] for h in range(NH)]
                else:
                    whs = [w_half(mo, h) for h in range(NH)]
                if mo == PRO and early_ps is not None:
                    psums = early_ps
                else:
                    psums = [
                        ps.tile([P, N_FREE], f32, tag="psum", name=f"psum_{mo}_{n}")
                        for n in range(NT)
                    ]
                for i in range(nk):
                    ip, k = divmod(i, KT)
                    h, kh = divmod(k, WH)
                    for n in range(NT):
                        nc.tensor.matmul(
                            psums[n][:],
                            whs[h][:, kh, :],
                            xt[i][:, n * N_FREE:(n + 1) * N_FREE],
                            start=(i == 0),
                            stop=(i == nk - 1),
                        )
                for n in range(NT):
                    evict(mo, n, psums[n])

    nc.compile()
    return nc


def _prep_inputs(x, weight_q, weight_scaler, mode):
    """Host-side shard + layout. Returns in_maps (list of dicts, one per core)."""
    xf = np.asarray(x, dtype=np.float32).reshape(TOKENS, IN_F)
    wq = np.asarray(weight_q)
    sc = np.asarray(weight_scaler, dtype=np.float32)

    # W tiles: w4[mo, p_in, ko, oc] = W[mo*128+oc, ko*128+p_in]
    # (matches the SBUF lhsT tile AP [P, KT, P] exactly), shipped as int8
    # and upconverted on-chip.
    w4 = np.ascontiguousarray(
        wq.reshape(MT, P, KT, P).transpose(0, 3, 2, 1)
    ).astype(np.int8)

    scal = np.ascontiguousarray(sc.reshape(MT, P).T)  # [P, MT]

    in_maps = []
    for c in range(N_CORES):
        xs = xf[c * T_SHARD:(c + 1) * T_SHARD, :]      # [T_SHARD, IN_F]
        xsT = np.ascontiguousarray(xs.T)                # [IN_F, T_SHARD] f32
        m = {"wq4": w4, "scal": scal}
        if mode == "f32r":
            m["xq0"] = xsT
        elif mode == "fp16":
            m["xq0"] = xsT.astype(np.float16)
        else:
            import ml_dtypes

            hi = xsT.astype(ml_dtypes.bfloat16)
            lo = (xsT - hi.astype(np.float32)).astype(ml_dtypes.bfloat16)
            m["xq0"] = hi
            m["xq1"] = lo
        in_maps.append(m)
    return in_maps


def _gather(results):
    """Per-core outT [MT, P, T_SHARD] -> full out [4, 2048, OUT_F] f32."""
    parts = []
    for c in range(N_CORES):
        ot = results[c]["outT"]                   # [MT, P, T_SHARD]
        parts.append(ot.reshape(OUT_F, T_SHARD).T)  # [T_SHARD, OUT_F]
    out = np.concatenate(parts, axis=0)           # [TOKENS, OUT_F]
    return np.ascontiguousarray(out.reshape(4, 2048, OUT_F), dtype=np.float32)


def _run(inputs, trace=False, mode=None):
    mode = mode or MODE
    if mode not in _cache:
        _cache[mode] = _build(mode)
    nc = _cache[mode]
    in_maps = _prep_inputs(inputs["x"], inputs["weight_q"], inputs["weight_scaler"], mode)
    res = run_bass_kernel_spmd(nc, in_maps, list(range(N_CORES)), trace=trace)
    return _gather(res.results), res


def kernel(**inputs):
    out, _ = _run(inputs, trace=False)
    return out


# revision 34
# speedup vs baseline: 1.0143x; 1.0143x over previous
"""Trainium2 Bass kernel for int8-dequant Linear: out = x @ (W_q * scaler)^T.

Full shapes: x [4, 2048, 4096] f32, weight_q [4096, 4096] int8,
weight_scaler [4096] f32 -> out [4, 2048, 4096] f32.

Sharding: data-parallel over tokens (8192 tokens -> 1024 per core);
weight_q/scaler replicated. Each core computes out.T for its token
shard with out-channels on PSUM partitions; the per-channel scaler is
applied as a per-partition scalar multiply on PSUM eviction.

Matmul dtype: float32r (TF32-like fast fp32 mode, 1 cyc/row at free
dim >= 256; measured rel-err ~2e-4 on K=4096 accumulation).
Fallback MODE "bf16x2": W exact in bf16 (int8-valued), x split into
bf16 hi+lo, two accumulation passes (rel err ~5e-6, 2x matmul work).
"""
import sys

sys.path.insert(0, "/opt/trn_rl_repo")

import numpy as np

import concourse.bacc as bacc
import concourse.mybir as mybir
import concourse.tile as tile
from concourse.bass_utils import run_bass_kernel_spmd

N_CORES = 8
P = 128
IN_F = 4096
OUT_F = 4096
TOKENS = 4 * 2048
T_SHARD = TOKENS // N_CORES          # 1024 tokens per core
KT = IN_F // P                       # 32 k-tiles
MT = OUT_F // P                      # 32 m-tiles (out-channel tiles)
N_FREE = 512                         # moving free dim per matmul (1 PSUM bank)
NT = T_SHARD // N_FREE               # 2 n-tiles

MODE = "f32r"                        # "f32r" | "fp16" | "bf16x2"

_cache = {}


def _build(mode):
    f32 = mybir.dt.float32
    mm_dt = {
        "f32r": mybir.dt.float32r,
        "fp16": mybir.dt.float16,
        "bf16x2": mybir.dt.bfloat16,
    }[mode]
    n_pass = 2 if mode == "bf16x2" else 1

    nc = bacc.Bacc(None, target_bir_lowering=False, debug=False)

    # DRAM parameters (per-core shapes)
    d_x = [
        nc.declare_dram_parameter(f"xq{i}", [IN_F, T_SHARD], mm_dt, isOutput=False)
        for i in range(n_pass)
    ]
    # Weights travel as int8 (4x less DMA + in-window bandwidth) and are
    # upconverted to the matmul dtype on-chip by the otherwise-idle DVE.
    d_w = nc.declare_dram_parameter("wq4", [MT, P, KT, P], mybir.dt.int8, isOutput=False)
    d_s = nc.declare_dram_parameter("scal", [P, MT], f32, isOutput=False)
    d_o = nc.declare_dram_parameter("outT", [MT, P, T_SHARD], f32, isOutput=True)

    WH = 8                       # k-tiles per weight quarter-tile
    NH = KT // WH                # weight sub-tiles per mo
    PRO = 4 if n_pass == 1 else 0  # mo-tiles interleaved during the x load

    with tile.TileContext(nc) as tc:
        with (
            tc.tile_pool(name="xp", bufs=KT * n_pass) as xp,
            tc.tile_pool(name="wp", bufs=12) as wp,
            tc.tile_pool(name="ws", bufs=6) as ws,
            tc.tile_pool(name="op", bufs=4) as op,
            tc.tile_pool(name="cp", bufs=1) as cp,
            tc.tile_pool(name="ps", bufs=8, space="PSUM") as ps,
        ):
            scal = cp.tile([P, MT], f32)
            nc.sync.dma_start(scal[:], d_s.ap())

            def w_half(mo, h):
                s = ws.tile([P, WH, P], mybir.dt.int8, tag="w8", name=f"w8_{mo}_{h}")
                nc.sync.dma_start(s[:], d_w.ap()[mo, :, h * WH:(h + 1) * WH, :])
                t = wp.tile([P, WH, P], mm_dt, tag="wh", name=f"wh_{mo}_{h}")
                nc.vector.tensor_copy(t[:], s[:])
                return t

            def x_tile(i):
                ip, k = divmod(i, KT)
                t = xp.tile([P, T_SHARD], mm_dt, tag="xt", name=f"xt_{i}")
                nc.sync.dma_start(t[:], d_x[ip].ap()[k * P:(k + 1) * P, :])
                return t

            def evict(mo, n, psum):
                osb = op.tile([P, N_FREE], f32, tag="osb", name=f"osb_{mo}_{n}")
                nc.vector.tensor_scalar_mul(osb[:], psum[:], scal[:, mo:mo + 1])
                nc.sync.dma_start(
                    d_o.ap()[mo, :, n * N_FREE:(n + 1) * N_FREE], osb[:]
                )

            nk = KT * n_pass
            xt = [None] * nk
            wh_pro = {}

            # --- phase 1: first PRO mo-tiles, k-major across mo so the PE
            # has work for every x k-tile as it lands. Each k-group's
            # weight quarters are emitted just ahead of that group's x
            # k-tiles; the last group also prefetches mo=PRO's quarters so
            # phase 2 starts without waiting behind the x tail.
            wh_next = {}
            if PRO:
                for h in range(NH):
                    if h == 0:
                        xt[0] = x_tile(0)
                        wh_pro[(0, 0)] = w_half(0, 0)
                        for mo in range(1, PRO):
                            wh_pro[(mo, 0)] = w_half(mo, 0)
                        for i in range(1, WH):
                            xt[i] = x_tile(i)
                    else:
                        for mo in range(PRO):
                            wh_pro[(mo, h)] = w_half(mo, h)
                        if h == NH - 1:
                            for j in range(NH):
                                wh_next[j] = w_half(PRO, j)
                        for i in range(h * WH, (h + 1) * WH):
                            xt[i] = x_tile(i)

                pro_ps = {
                    (mo, n): ps.tile(
                        [P, N_FREE], f32, tag="psum", name=f"psum_{mo}_{n}"
                    )
                    for mo in range(PRO)
                    for n in range(NT)
                }
                # Request mo=PRO's banks now so the allocator binds them to
                # the earliest-released phase-1 banks (shrinks the
                # phase-boundary stall).
                early_ps = [
                    ps.tile([P, N_FREE], f32, tag="psum", name=f"psum_{PRO}_{n}")
                    for n in range(NT)
                ]
                for i in range(nk):
                    ip, k = divmod(i, KT)
                    h, kh = divmod(k, WH)
                    for mo in range(PRO):
                        for n in range(NT):
                            nc.tensor.matmul(
                                pro_ps[(mo, n)][:],
                                wh_pro[(mo, h)][:, kh, :],
                                xt[i][:, n * N_FREE:(n + 1) * N_FREE],
                                start=(i == 0),
                                stop=(i == nk - 1),
                            )
                for mo in range(PRO):
                    for n in range(NT):
                        evict(mo, n, pro_ps[(mo, n)])
            else:
                early_ps = None
                for i in range(nk):
                    xt[i] = x_tile(i)

            # --- phase 2: remaining mo-tiles, weight-reuse-friendly order
            # (k middle, n inner).
            for mo in range(PRO, MT):
                if mo == PRO and wh_next:
                    whs = [wh_next[h] for h in range(NH)]
                else:
                    whs = [w_half(mo, h) for h in range(NH)]
                if mo == PRO and early_ps is not None:
                    psums = early_ps
                else:
                    psums = [
                        ps.tile([P, N_FREE], f32, tag="psum", name=f"psum_{mo}_{n}")
                        for n in range(NT)
                    ]
                for i in range(nk):
                    ip, k = divmod(i, KT)
                    h, kh = divmod(k, WH)
                    for n in range(NT):
                        nc.tensor.matmul(
                            psums[n][:],
                            whs[h][:, kh, :],
                            xt[i][:, n * N_FREE:(n + 1) * N_FREE],
                            start=(i == 0),
                            stop=(i == nk - 1),
                        )
                for n in range(NT):
                    evict(mo, n, psums[n])

    nc.compile()
    return nc


def _prep_inputs(x, weight_q, weight_scaler, mode):
    """Host-side shard + layout. Returns in_maps (list of dicts, one per core)."""
    xf = np.asarray(x, dtype=np.float32).reshape(TOKENS, IN_F)
    wq = np.asarray(weight_q)
    sc = np.asarray(weight_scaler, dtype=np.float32)

    # W tiles: w4[mo, p_in, ko, oc] = W[mo*128+oc, ko*128+p_in]
    # (matches the SBUF lhsT tile AP [P, KT, P] exactly), shipped as int8
    # and upconverted on-chip.
    w4 = np.ascontiguousarray(
        wq.reshape(MT, P, KT, P).transpose(0, 3, 2, 1)
    ).astype(np.int8)

    scal = np.ascontiguousarray(sc.reshape(MT, P).T)  # [P, MT]

    in_maps = []
    for c in range(N_CORES):
        xs = xf[c * T_SHARD:(c + 1) * T_SHARD, :]      # [T_SHARD, IN_F]
        xsT = np.ascontiguousarray(xs.T)                # [IN_F, T_SHARD] f32
        m = {"wq4": w4, "scal": scal}
        if mode == "f32r":
            m["xq0"] = xsT
        elif mode == "fp16":
            m["xq0"] = xsT.astype(np.float16)
        else:
            import ml_dtypes

            hi = xsT.astype(ml_dtypes.bfloat16)
            lo = (xsT - hi.astype(np.float32)).astype(ml_dtypes.bfloat16)
            m["xq0"] = hi
            m["xq1"] = lo
        in_maps.append(m)
    return in_maps


def _gather(results):
    """Per-core outT [MT, P, T_SHARD] -> full out [4, 2048, OUT_F] f32."""
    parts = []
    for c in range(N_CORES):
        ot = results[c]["outT"]                   # [MT, P, T_SHARD]
        parts.append(ot.reshape(OUT_F, T_SHARD).T)  # [T_SHARD, OUT_F]
    out = np.concatenate(parts, axis=0)           # [TOKENS, OUT_F]
    return np.ascontiguousarray(out.reshape(4, 2048, OUT_F), dtype=np.float32)


def _run(inputs, trace=False, mode=None):
    mode = mode or MODE
    if mode not in _cache:
        _cache[mode] = _build(mode)
    nc = _cache[mode]
    in_maps = _prep_inputs(inputs["x"], inputs["weight_q"], inputs["weight_scaler"], mode)
    res = run_bass_kernel_spmd(nc, in_maps, list(range(N_CORES)), trace=trace)
    return _gather(res.results), res


def kernel(**inputs):
    out, _ = _run(inputs, trace=False)
    return out


# revision 35
# speedup vs baseline: 1.0190x; 1.0046x over previous
"""Trainium2 Bass kernel for int8-dequant Linear: out = x @ (W_q * scaler)^T.

Full shapes: x [4, 2048, 4096] f32, weight_q [4096, 4096] int8,
weight_scaler [4096] f32 -> out [4, 2048, 4096] f32.

Sharding: data-parallel over tokens (8192 tokens -> 1024 per core);
weight_q/scaler replicated. Each core computes out.T for its token
shard with out-channels on PSUM partitions; the per-channel scaler is
applied as a per-partition scalar multiply on PSUM eviction.

Matmul dtype: float32r (TF32-like fast fp32 mode, 1 cyc/row at free
dim >= 256; measured rel-err ~2e-4 on K=4096 accumulation).
Fallback MODE "bf16x2": W exact in bf16 (int8-valued), x split into
bf16 hi+lo, two accumulation passes (rel err ~5e-6, 2x matmul work).
"""
import sys

sys.path.insert(0, "/opt/trn_rl_repo")

import numpy as np

import concourse.bacc as bacc
import concourse.mybir as mybir
import concourse.tile as tile
from concourse.bass_utils import run_bass_kernel_spmd

N_CORES = 8
P = 128
IN_F = 4096
OUT_F = 4096
TOKENS = 4 * 2048
T_SHARD = TOKENS // N_CORES          # 1024 tokens per core
KT = IN_F // P                       # 32 k-tiles
MT = OUT_F // P                      # 32 m-tiles (out-channel tiles)
N_FREE = 512                         # moving free dim per matmul (1 PSUM bank)
NT = T_SHARD // N_FREE               # 2 n-tiles

MODE = "f32r"                        # "f32r" | "fp16" | "bf16x2"

_cache = {}


def _build(mode):
    f32 = mybir.dt.float32
    mm_dt = {
        "f32r": mybir.dt.float32r,
        "fp16": mybir.dt.float16,
        "bf16x2": mybir.dt.bfloat16,
    }[mode]
    n_pass = 2 if mode == "bf16x2" else 1

    nc = bacc.Bacc(None, target_bir_lowering=False, debug=False)

    # DRAM parameters (per-core shapes)
    d_x = [
        nc.declare_dram_parameter(f"xq{i}", [IN_F, T_SHARD], mm_dt, isOutput=False)
        for i in range(n_pass)
    ]
    # Weights travel as int8 (4x less DMA + in-window bandwidth) and are
    # upconverted to the matmul dtype on-chip by the otherwise-idle DVE.
    d_w = nc.declare_dram_parameter("wq4", [MT, P, KT, P], mybir.dt.int8, isOutput=False)
    d_s = nc.declare_dram_parameter("scal", [P, MT], f32, isOutput=False)
    d_o = nc.declare_dram_parameter("outT", [MT, P, T_SHARD], f32, isOutput=True)

    WH = 8                       # k-tiles per weight quarter-tile
    NH = KT // WH                # weight sub-tiles per mo
    PRO = 4 if n_pass == 1 else 0  # mo-tiles interleaved during the x load

    with tile.TileContext(nc) as tc:
        with (
            tc.tile_pool(name="xp", bufs=KT * n_pass) as xp,
            tc.tile_pool(name="wp", bufs=12) as wp,
            tc.tile_pool(name="ws", bufs=6) as ws,
            tc.tile_pool(name="op", bufs=4) as op,
            tc.tile_pool(name="cp", bufs=1) as cp,
            tc.tile_pool(name="ps", bufs=8, space="PSUM") as ps,
        ):
            scal = cp.tile([P, MT], f32)
            nc.sync.dma_start(scal[:], d_s.ap())

            def w_half(mo, h):
                s = ws.tile([P, WH, P], mybir.dt.int8, tag="w8", name=f"w8_{mo}_{h}")
                nc.sync.dma_start(s[:], d_w.ap()[mo, :, h * WH:(h + 1) * WH, :])
                t = wp.tile([P, WH, P], mm_dt, tag="wh", name=f"wh_{mo}_{h}")
                nc.vector.tensor_copy(t[:], s[:])
                return t

            def x_tile(i):
                ip, k = divmod(i, KT)
                t = xp.tile([P, T_SHARD], mm_dt, tag="xt", name=f"xt_{i}")
                nc.sync.dma_start(t[:], d_x[ip].ap()[k * P:(k + 1) * P, :])
                return t

            def evict(mo, n, psum):
                osb = op.tile([P, N_FREE], f32, tag="osb", name=f"osb_{mo}_{n}")
                nc.vector.tensor_scalar_mul(osb[:], psum[:], scal[:, mo:mo + 1])
                nc.sync.dma_start(
                    d_o.ap()[mo, :, n * N_FREE:(n + 1) * N_FREE], osb[:]
                )

            nk = KT * n_pass
            xt = [None] * nk
            wh_pro = {}

            # --- phase 1: first PRO mo-tiles, k-major across mo so the PE
            # has work for every x k-tile as it lands. Each k-group's
            # weight quarters are emitted just ahead of that group's x
            # k-tiles; the last group also prefetches mo=PRO's quarters so
            # phase 2 starts without waiting behind the x tail.
            wh_next = {}
            if PRO:
                for h in range(NH):
                    if h == 0:
                        # int8 quarters are tiny (128KB): emit all four
                        # before x0 so their DVE conversions complete during
                        # x0's transfer instead of serializing after it.
                        for mo in range(PRO):
                            wh_pro[(mo, 0)] = w_half(mo, 0)
                        for i in range(0, WH):
                            xt[i] = x_tile(i)
                    else:
                        for mo in range(PRO):
                            wh_pro[(mo, h)] = w_half(mo, h)
                        if h == NH - 1:
                            for j in range(NH):
                                wh_next[j] = w_half(PRO, j)
                        for i in range(h * WH, (h + 1) * WH):
                            xt[i] = x_tile(i)

                pro_ps = {
                    (mo, n): ps.tile(
                        [P, N_FREE], f32, tag="psum", name=f"psum_{mo}_{n}"
                    )
                    for mo in range(PRO)
                    for n in range(NT)
                }
                # Request mo=PRO's banks now so the allocator binds them to
                # the earliest-released phase-1 banks (shrinks the
                # phase-boundary stall).
                early_ps = [
                    ps.tile([P, N_FREE], f32, tag="psum", name=f"psum_{PRO}_{n}")
                    for n in range(NT)
                ]
                for i in range(nk):
                    ip, k = divmod(i, KT)
                    h, kh = divmod(k, WH)
                    for mo in range(PRO):
                        for n in range(NT):
                            nc.tensor.matmul(
                                pro_ps[(mo, n)][:],
                                wh_pro[(mo, h)][:, kh, :],
                                xt[i][:, n * N_FREE:(n + 1) * N_FREE],
                                start=(i == 0),
                                stop=(i == nk - 1),
                            )
                for mo in range(PRO):
                    for n in range(NT):
                        evict(mo, n, pro_ps[(mo, n)])
            else:
                early_ps = None
                for i in range(nk):
                    xt[i] = x_tile(i)

            # --- phase 2: remaining mo-tiles, weight-reuse-friendly order
            # (k middle, n inner).
            for mo in range(PRO, MT):
                if mo == PRO and wh_next:
                    whs = [wh_next[h] for h in range(NH)]
                else:
                    whs = [w_half(mo, h) for h in range(NH)]
                if mo == PRO and early_ps is not None:
                    psums = early_ps
                else:
                    psums = [
                        ps.tile([P, N_FREE], f32, tag="psum", name=f"psum_{mo}_{n}")
                        for n in range(NT)
                    ]
                for i in range(nk):
                    ip, k = divmod(i, KT)
                    h, kh = divmod(k, WH)
                    for n in range(NT):
                        nc.tensor.matmul(
                            psums[n][:],
                            whs[h][:, kh, :],
                            xt[i][:, n * N_FREE:(n + 1) * N_FREE],
                            start=(i == 0),
                            stop=(i == nk - 1),
                        )
                for n in range(NT):
                    evict(mo, n, psums[n])

    nc.compile()
    return nc


def _prep_inputs(x, weight_q, weight_scaler, mode):
    """Host-side shard + layout. Returns in_maps (list of dicts, one per core)."""
    xf = np.asarray(x, dtype=np.float32).reshape(TOKENS, IN_F)
    wq = np.asarray(weight_q)
    sc = np.asarray(weight_scaler, dtype=np.float32)

    # W tiles: w4[mo, p_in, ko, oc] = W[mo*128+oc, ko*128+p_in]
    # (matches the SBUF lhsT tile AP [P, KT, P] exactly), shipped as int8
    # and upconverted on-chip.
    w4 = np.ascontiguousarray(
        wq.reshape(MT, P, KT, P).transpose(0, 3, 2, 1)
    ).astype(np.int8)

    scal = np.ascontiguousarray(sc.reshape(MT, P).T)  # [P, MT]

    in_maps = []
    for c in range(N_CORES):
        xs = xf[c * T_SHARD:(c + 1) * T_SHARD, :]      # [T_SHARD, IN_F]
        xsT = np.ascontiguousarray(xs.T)                # [IN_F, T_SHARD] f32
        m = {"wq4": w4, "scal": scal}
        if mode == "f32r":
            m["xq0"] = xsT
        elif mode == "fp16":
            m["xq0"] = xsT.astype(np.float16)
        else:
            import ml_dtypes

            hi = xsT.astype(ml_dtypes.bfloat16)
            lo = (xsT - hi.astype(np.float32)).astype(ml_dtypes.bfloat16)
            m["xq0"] = hi
            m["xq1"] = lo
        in_maps.append(m)
    return in_maps


def _gather(results):
    """Per-core outT [MT, P, T_SHARD] -> full out [4, 2048, OUT_F] f32."""
    parts = []
    for c in range(N_CORES):
        ot = results[c]["outT"]                   # [MT, P, T_SHARD]
        parts.append(ot.reshape(OUT_F, T_SHARD).T)  # [T_SHARD, OUT_F]
    out = np.concatenate(parts, axis=0)           # [TOKENS, OUT_F]
    return np.ascontiguousarray(out.reshape(4, 2048, OUT_F), dtype=np.float32)


def _run(inputs, trace=False, mode=None):
    mode = mode or MODE
    if mode not in _cache:
        _cache[mode] = _build(mode)
    nc = _cache[mode]
    in_maps = _prep_inputs(inputs["x"], inputs["weight_q"], inputs["weight_scaler"], mode)
    res = run_bass_kernel_spmd(nc, in_maps, list(range(N_CORES)), trace=trace)
    return _gather(res.results), res


def kernel(**inputs):
    out, _ = _run(inputs, trace=False)
    return out
